# revision 6
# baseline (speedup 1.0000x reference)
"""Mixtral sparse MoE block on 8 Trainium2 NeuronCores.

Strategy: F-sharded tensor parallelism (perfect load balance). Each core
holds a 512-wide slice of the FFN dim of ALL 8 experts' weights (bf16).
The host routes tokens into expert-pure segments (<= 512 tokens each, the
PSUM bank limit); every core processes ALL T*K = 4096 token-expert pairs
over its F-slice:

    h[fsl] = silu(x @ w1[:, fsl]) * (x @ w2[:, fsl])   (phase A)
    y_partial = h[fsl] @ w3[fsl, :]                    (phase B)

and the host sums the 8 partial outputs, applies the routing gates, and
scatter-adds into the [T, H] output. Per-core compute is exactly
T*K*3*H*F/8 MACs regardless of routing imbalance, and the whole block is
a single SPMD dispatch (no token-capacity overflow rounds).

Everything streams in bf16 (weights, activations, h), halving HBM traffic
vs fp32r; PSUM accumulation is fp32. Measured end-to-end l2 error ~4e-3
vs the fp32 reference (gate is 2e-2). Matmuls are issued in software-
pipelined order A(s0), A(s1), B(s0), A(s2), B(s1), ... so the tensor
engine never waits on the scalar/vector silu*mul producing h.
"""

import sys

for _p in ("/opt/trn_rl_repo", "/root/.axon_site/_ro/trn_rl_repo"):
    if _p not in sys.path:
        sys.path.append(_p)

import numpy as np

H = 2048   # hidden dim
F = 4096   # ffn dim
E = 8      # experts
NC = 8     # cores
FL = F // NC          # per-core F slice (512)
FLT = FL // 128       # f tiles per core (4)
HT = H // 128         # h tiles (16)
SEG_MAX = 512         # PSUM bank: 512 fp32 per partition

_COMPILED = {}

# set by a driver (e.g. test.py) to profile the next dispatch
TRACE = False
LAST_EXEC_NS = None
LAST_RESULTS = None


def _ensure_ntff_hook():
    """Install antenv.axon_hooks shim + register the axon NTFF profile hook
    if the image's antenv package lacks it. Only needed for TRACE runs."""
    try:
        from antenv import axon_hooks  # noqa: F401
        return
    except ImportError:
        pass
    import types
    import antenv

    mod = types.ModuleType("antenv.axon_hooks")
    _hook = [None]
    mod.set_axon_ntff_profile_hook = lambda h: _hook.__setitem__(0, h)
    mod.get_axon_ntff_profile_hook = lambda: _hook[0]
    sys.modules["antenv.axon_hooks"] = mod
    antenv.axon_hooks = mod
    try:
        from trn_agent_boot.trn_boot import _ntff_profile_via_ctypes
        mod.set_axon_ntff_profile_hook(
            _ntff_profile_via_ctypes("/opt/axon/libaxon_pjrt.so")
        )
    except Exception:
        pass


def _plan_segments(counts):
    """Expert-pure token segments, each <= SEG_MAX. Smallest expert first
    (fastest DMA ramp to the first matmul chain)."""
    segs = []
    for e in np.argsort(counts):
        n = int(counts[e])
        if n == 0:
            continue
        k = -(-n // SEG_MAX)
        base, rem = n // k, n % k
        for i in range(k):
            segs.append((int(e), base + (1 if i < rem else 0)))
    return tuple(segs)


def _build(segs):
    import concourse.bacc as bacc
    import concourse.tile as tile
    from concourse import mybir

    F32 = mybir.dt.float32
    BF16 = mybir.dt.bfloat16

    tot = sum(L for _, L in segs)        # total token-expert pairs (4096)
    XW = HT * tot                        # xg/yT flat cols

    nc = bacc.Bacc("TRN2", target_bir_lowering=False, debug=False, num_devices=NC)
    # flat per-partition-contiguous layouts (see kernel() for host packing):
    #   xg[p, seg: t*L+c]        = x[tok_c, t*128+p]          (bf16)
    #   w1[p, e, (fo*16+t)*128+j] = w1[e][t*128+p, c*FL+fo*128+j]
    #   w2 identical; w3[p, e, (fo*16+t)*128+j] = w3[e][c*FL+fo*128+p, t*128+j]
    #   yT[p, seg: t*L+c]        = y_partial[tok_c, t*128+p]  (f32)
    xg = nc.dram_tensor("xg", [128, XW], BF16, kind="ExternalInput").ap()
    w1 = nc.dram_tensor("w1", [128, E * FLT * HT * 128], BF16, kind="ExternalInput").ap()
    w2 = nc.dram_tensor("w2", [128, E * FLT * HT * 128], BF16, kind="ExternalInput").ap()
    w3 = nc.dram_tensor("w3", [128, E * FLT * HT * 128], BF16, kind="ExternalInput").ap()
    wz = nc.dram_tensor("wz", [128, 256], BF16, kind="ExternalInput").ap()
    yT = nc.dram_tensor("yT", [128, XW], F32, kind="ExternalOutput").ap()

    EW = FLT * HT * 128                  # per-expert flat weight cols (8192)

    with tile.TileContext(nc) as tc:
        with (
            tc.tile_pool(name="w12pool", bufs=2) as w12pool,
            tc.tile_pool(name="w3pool", bufs=2) as w3pool,
            tc.tile_pool(name="xpool", bufs=2) as xpool,
            tc.tile_pool(name="hpool", bufs=2) as hpool,
            tc.tile_pool(name="spool", bufs=3) as spool,
            tc.tile_pool(name="ypool", bufs=3) as ypool,
            tc.tile_pool(name="resident", bufs=1) as resident,
            tc.tile_pool(name="psA", bufs=2, space="PSUM") as psA,
            tc.tile_pool(name="psY", bufs=3, space="PSUM") as psY,
            tc.tile_pool(name="psW", bufs=1, space="PSUM") as psW,
        ):
            # HAM warmup: dummy bf16 matmuls on a tiny zeros input run
            # while the first token/weight DMAs stream in, so the PE
            # clock-gate is already released when real matmuls start.
            warm = resident.tile([128, 256], BF16)
            nc.sync.dma_start(warm[:], wz[:])
            for i in range(40):
                pw = psW.tile([128, 256], F32, tag="pw")
                nc.tensor.matmul(
                    pw[:], warm[:, :128], warm[:], start=True, stop=True
                )

            # per-expert weight tiles, loaded once per expert (segments of
            # one expert are consecutive in `segs`)
            wtiles = {}

            def load_expert(e, first):
                # w1/w2 live until the expert's last A; w3 until its last B.
                # Separate pools so the next expert's prefetch isn't gated
                # on this expert's B-phase retiring.
                w1t = w12pool.tile([128, EW], BF16, tag="w1t")
                w2t = w12pool.tile([128, EW], BF16, tag="w2t")
                w3t = w3pool.tile([128, EW], BF16, tag="w3t")
                o = e * EW
                if first:
                    # ramp: split w1 so the first chain's fo=0 slice lands
                    # quickly
                    q = EW // 4
                    nc.sync.dma_start(w1t[:, :q], w1[:, o:o + q])
                    nc.sync.dma_start(w1t[:, q:], w1[:, o + q:o + EW])
                else:
                    nc.sync.dma_start(w1t[:], w1[:, o:o + EW])
                nc.sync.dma_start(w2t[:], w2[:, o:o + EW])
                nc.sync.dma_start(w3t[:], w3[:, o:o + EW])
                return w1t, w2t, w3t

            def phase_a(si):
                e, L, off = segs2[si]
                if e not in wtiles:
                    wtiles[e] = load_expert(e, first=(si == 0))
                w1t, w2t, _ = wtiles[e]
                xs = xpool.tile([128, HT * SEG_MAX], BF16, tag="xs")
                if si == 0:
                    hh = (HT // 2) * L
                    nc.sync.dma_start(xs[:, :hh], xg[:, off:off + hh])
                    nc.sync.dma_start(xs[:, hh:HT * L], xg[:, off + hh:off + HT * L])
                else:
                    nc.sync.dma_start(xs[:, :HT * L], xg[:, off:off + HT * L])
                h = hpool.tile([128, FLT * SEG_MAX], BF16, tag="h")
                for fo in range(FLT):
                    pa = psA.tile([128, SEG_MAX], F32, tag="pa")
                    for t in range(HT):
                        nc.tensor.matmul(
                            pa[:, :L],
                            w1t[:, (fo * HT + t) * 128:(fo * HT + t + 1) * 128],
                            xs[:, t * L:(t + 1) * L],
                            start=(t == 0), stop=(t == HT - 1),
                        )
                    pb = psA.tile([128, SEG_MAX], F32, tag="pb")
                    for t in range(HT):
                        nc.tensor.matmul(
                            pb[:, :L],
                            w2t[:, (fo * HT + t) * 128:(fo * HT + t + 1) * 128],
                            xs[:, t * L:(t + 1) * L],
                            start=(t == 0), stop=(t == HT - 1),
                        )
                    sa = spool.tile([128, SEG_MAX], F32, tag="sa")
                    nc.scalar.activation(
                        sa[:, :L], pa[:, :L], mybir.ActivationFunctionType.Silu
                    )
                    nc.vector.tensor_mul(
                        h[:, fo * L:(fo + 1) * L], sa[:, :L], pb[:, :L]
                    )
                hsb[si] = h

            def phase_b(si):
                e, L, off = segs2[si]
                w3t = wtiles[e][2]
                h = hsb.pop(si)
                for t in range(HT):
                    py = psY.tile([128, SEG_MAX], F32, tag="py")
                    for fo in range(FLT):
                        nc.tensor.matmul(
                            py[:, :L],
                            w3t[:, (fo * HT + t) * 128:(fo * HT + t + 1) * 128],
                            h[:, fo * L:(fo + 1) * L],
                            start=(fo == 0), stop=(fo == FLT - 1),
                        )
                    yt = ypool.tile([128, SEG_MAX], F32, tag="yt")
                    nc.vector.tensor_copy(yt[:, :L], py[:, :L])
                    nc.sync.dma_start(yT[:, off + t * L:off + (t + 1) * L], yt[:, :L])

            # segment offsets into the flat xg/yT streams
            segs2 = []
            off = 0
            for e, L in segs:
                segs2.append((e, L, off))
                off += HT * L
            hsb = {}

            # software pipeline: A(0), A(1), B(0), A(2), B(1), ...
            n = len(segs2)
            phase_a(0)
            for si in range(1, n):
                phase_a(si)
                phase_b(si - 1)
            phase_b(n - 1)

    nc.compile()
    return nc


def _get_compiled(segs):
    if segs not in _COMPILED:
        _COMPILED[segs] = _build(segs)
    return _COMPILED[segs]


def kernel(hidden_states, selected_experts, routing_weights, w1, w2, w3):
    global LAST_EXEC_NS, LAST_RESULTS
    import ml_dtypes
    from concourse.bass_utils import run_bass_kernel_spmd

    BF = ml_dtypes.bfloat16
    hs = np.ascontiguousarray(np.asarray(hidden_states), dtype=np.float32)
    sel = np.asarray(selected_experts)
    rw = np.ascontiguousarray(np.asarray(routing_weights), dtype=np.float32)
    w1 = np.asarray(w1, dtype=np.float32)
    w2 = np.asarray(w2, dtype=np.float32)
    w3 = np.asarray(w3, dtype=np.float32)

    T = hs.shape[0]
    K = sel.shape[1]
    assert hs.shape[1] == H and w1.shape == (E, H, F) and w3.shape == (E, F, H)

    # host routing: gate[t, e] = sum_k rw[t, k] * (sel[t, k] == e)
    gate = np.zeros((T, E), np.float32)
    member = np.zeros((T, E), bool)
    tix = np.arange(T)
    for k in range(K):
        np.add.at(gate, (tix, sel[:, k]), rw[:, k])
        member[tix, sel[:, k]] = True
    idx = [np.nonzero(member[:, e])[0] for e in range(E)]
    counts = np.array([len(i) for i in idx])

    segs = _plan_segments(counts)
    tot = sum(L for _, L in segs)
    XW = HT * tot

    # pack the token stream: expert-pure segments, transposed to
    # [128 (h-within-tile), HT, L] and flattened per partition
    xr = hs.astype(BF)
    xgf = np.zeros((128, XW), BF)
    seg_meta = []  # (expert, token_indices, col_offset)
    used = {e: 0 for e in range(E)}
    off = 0
    for e, L in segs:
        tt = idx[e][used[e]:used[e] + L]
        used[e] += L
        blk = xr[tt].reshape(L, HT, 128).transpose(2, 1, 0)  # [128, HT, L]
        xgf[:, off:off + HT * L] = blk.reshape(128, HT * L)
        seg_meta.append((e, tt, off))
        off += HT * L

    # per-core F-sliced weights, bf16, per-partition-contiguous
    EW = FLT * HT * 128
    in_maps = []
    wz0 = np.zeros((128, 256), BF)
    for c in range(NC):
        fsl = slice(c * FL, (c + 1) * FL)
        w1p = np.empty((128, E * EW), BF)
        w2p = np.empty((128, E * EW), BF)
        w3p = np.empty((128, E * EW), BF)
        for e in range(E):
            # stat[p, (fo*HT+t)*128+j] = w1[e][t*128+p, c*FL+fo*128+j]
            w1p[:, e * EW:(e + 1) * EW] = (
                w1[e][:, fsl].astype(BF)
                .reshape(HT, 128, FLT, 128).transpose(1, 2, 0, 3)
                .reshape(128, EW)
            )
            w2p[:, e * EW:(e + 1) * EW] = (
                w2[e][:, fsl].astype(BF)
                .reshape(HT, 128, FLT, 128).transpose(1, 2, 0, 3)
                .reshape(128, EW)
            )
            # stat[p, (fo*HT+t)*128+j] = w3[e][c*FL+fo*128+p, t*128+j]
            w3p[:, e * EW:(e + 1) * EW] = (
                w3[e][fsl, :].astype(BF)
                .reshape(FLT, 128, HT, 128).transpose(1, 0, 2, 3)
                .reshape(128, EW)
            )
        in_maps.append({"xg": xgf, "w1": w1p, "w2": w2p, "w3": w3p, "wz": wz0})

    if TRACE:
        _ensure_ntff_hook()
    nc = _get_compiled(segs)
    res = run_bass_kernel_spmd(
        nc, in_maps, core_ids=list(range(NC)),
        trace=TRACE, trace_cores=(list(range(NC)) if TRACE else None),
    )
    if TRACE:
        LAST_EXEC_NS = res.exec_time_ns
        LAST_RESULTS = res

    # sum the 8 F-slice partials, then gate + scatter-add on the host
    ysum = res.results[0]["yT"].astype(np.float64)
    for c in range(1, NC):
        ysum += res.results[c]["yT"]
    ysum = ysum.astype(np.float32)

    out = np.zeros((T, H), np.float32)
    for e, tt, off in seg_meta:
        L = len(tt)
        y = (
            ysum[:, off:off + HT * L].reshape(128, HT, L)
            .transpose(2, 1, 0).reshape(L, H)
        )
        out[tt] += gate[tt, e:e + 1] * y
    return out


# revision 14
# speedup vs baseline: 1.2030x; 1.2030x over previous
"""Mixtral sparse MoE block on 8 Trainium2 NeuronCores.

Strategy: F-sharded tensor parallelism (perfect load balance). Each core
holds a 512-wide slice of the FFN dim of ALL 8 experts' weights (bf16).
The host routes tokens into expert-pure segments (<= 512 tokens each, the
PSUM bank limit); every core processes ALL T*K = 4096 token-expert pairs
over its F-slice:

    h[fsl] = silu(x @ w1[:, fsl]) * (x @ w2[:, fsl])   (phase A)
    y_partial = h[fsl] @ w3[fsl, :]                    (phase B)

and the host sums the 8 partial outputs, applies the routing gates, and
scatter-adds into the [T, H] output. Per-core compute is exactly
T*K*3*H*F/8 MACs regardless of routing imbalance, and the whole block is
a single SPMD dispatch (no token-capacity overflow rounds).

Everything streams in bf16 (weights, activations, h), halving HBM traffic
vs fp32r; PSUM accumulation is fp32. Measured end-to-end l2 error ~4e-3
vs the fp32 reference (gate is 2e-2). Matmuls are issued in software-
pipelined order A(s0), A(s1), B(s0), A(s2), B(s1), ... so the tensor
engine never waits on the scalar/vector silu*mul producing h.
"""

import sys

for _p in ("/opt/trn_rl_repo", "/root/.axon_site/_ro/trn_rl_repo"):
    if _p not in sys.path:
        sys.path.append(_p)

import numpy as np

H = 2048   # hidden dim
F = 4096   # ffn dim
E = 8      # experts
NC = 8     # cores
FL = F // NC          # per-core F slice (512)
FLT = FL // 128       # f tiles per core (4)
HT = H // 128         # h tiles (16)
SEG_MAX = 512         # PSUM bank: 512 fp32 per partition

_COMPILED = {}

# set by a driver (e.g. test.py) to profile the next dispatch
TRACE = False
LAST_EXEC_NS = None
LAST_RESULTS = None


def _ensure_ntff_hook():
    """Install antenv.axon_hooks shim + register the axon NTFF profile hook
    if the image's antenv package lacks it. Only needed for TRACE runs."""
    try:
        from antenv import axon_hooks  # noqa: F401
        return
    except ImportError:
        pass
    import types
    import antenv

    mod = types.ModuleType("antenv.axon_hooks")
    _hook = [None]
    mod.set_axon_ntff_profile_hook = lambda h: _hook.__setitem__(0, h)
    mod.get_axon_ntff_profile_hook = lambda: _hook[0]
    sys.modules["antenv.axon_hooks"] = mod
    antenv.axon_hooks = mod
    try:
        from trn_agent_boot.trn_boot import _ntff_profile_via_ctypes
        mod.set_axon_ntff_profile_hook(
            _ntff_profile_via_ctypes("/opt/axon/libaxon_pjrt.so")
        )
    except Exception:
        pass


def _plan_segments(counts):
    """Expert-pure token segments, each <= SEG_MAX, padded to a multiple of
    16 (32B-aligned bf16 SBUF offsets; odd lengths cost ~20ns extra per
    matmul). Sorted ascending so the smallest segment leads (fast DMA ramp)
    and the largest trail (the back-to-back B phases at the tail need
    per-t PE time > the vector copy time to avoid PSUM recycling stalls).
    Returns (expert, L_real, L_padded) triples; expert segments stay
    consecutive after sorting since per-expert segment sizes are equal."""
    segs = []
    for e in range(len(counts)):
        n = int(counts[e])
        if n == 0:
            continue
        k = -(-n // SEG_MAX)
        base, rem = n // k, n % k
        for i in range(k):
            L = base + (1 if i < rem else 0)
            segs.append((int(e), L, min(SEG_MAX, -(-L // 16) * 16)))
    ekey = {}
    for e, L, Lp in segs:
        ekey[e] = max(ekey.get(e, 0), Lp)
    segs.sort(key=lambda s: (ekey[s[0]], s[0]))
    return tuple(segs)


def _build(segs):
    import concourse.bacc as bacc
    import concourse.tile as tile
    from concourse import mybir

    F32 = mybir.dt.float32
    BF16 = mybir.dt.bfloat16

    tot = sum(Lp for _, _, Lp in segs)   # padded token-expert pairs
    XW = HT * tot                        # xg/yT flat cols

    nc = bacc.Bacc("TRN2", target_bir_lowering=False, debug=False, num_devices=NC)
    # flat per-partition-contiguous layouts (see kernel() for host packing):
    #   xg[p, seg: t*L+c]        = x[tok_c, t*128+p]          (bf16)
    #   w1[p, e, (fo*16+t)*128+j] = w1[e][t*128+p, c*FL+fo*128+j]
    #   w2 identical; w3[p, e, (fo*16+t)*128+j] = w3[e][c*FL+fo*128+p, t*128+j]
    #   yT[p, seg: t*L+c]        = y_partial[tok_c, t*128+p]  (f32)
    xg = nc.dram_tensor("xg", [128, XW], BF16, kind="ExternalInput").ap()
    w1 = nc.dram_tensor("w1", [128, E * FLT * HT * 128], BF16, kind="ExternalInput").ap()
    w2 = nc.dram_tensor("w2", [128, E * FLT * HT * 128], BF16, kind="ExternalInput").ap()
    w3 = nc.dram_tensor("w3", [128, E * FLT * HT * 128], BF16, kind="ExternalInput").ap()
    wz = nc.dram_tensor("wz", [128, 256], BF16, kind="ExternalInput").ap()
    yT = nc.dram_tensor("yT", [128, XW], F32, kind="ExternalOutput").ap()

    EW = FLT * HT * 128                  # per-expert flat weight cols (8192)

    with tile.TileContext(nc) as tc:
        with (
            tc.tile_pool(name="w12pool", bufs=2) as w12pool,
            tc.tile_pool(name="w3pool", bufs=2) as w3pool,
            tc.tile_pool(name="xpool", bufs=2) as xpool,
            tc.tile_pool(name="hpool", bufs=2) as hpool,
            tc.tile_pool(name="spool", bufs=3) as spool,
            tc.tile_pool(name="ypool", bufs=3) as ypool,
            tc.tile_pool(name="resident", bufs=1) as resident,
            tc.tile_pool(name="psA", bufs=2, space="PSUM") as psA,
            tc.tile_pool(name="psY", bufs=3, space="PSUM") as psY,
            tc.tile_pool(name="psW", bufs=1, space="PSUM") as psW,
        ):
            # HAM warmup: dummy bf16 matmuls on a tiny zeros input run
            # while the first token/weight DMAs stream in, so the PE
            # clock-gate is already released when real matmuls start.
            warm = resident.tile([128, 256], BF16)
            nc.sync.dma_start(warm[:], wz[:])
            for i in range(24):
                pw = psW.tile([128, 256], F32, tag="pw")
                nc.tensor.matmul(
                    pw[:], warm[:, :128], warm[:], start=True, stop=True
                )

            # per-expert weight tiles, loaded once per expert (segments of
            # one expert are consecutive in `segs`)
            wtiles = {}

            def load_expert(e, first):
                # w1/w2 live until the expert's last A; w3 until its last B.
                # Separate pools so the next expert's prefetch isn't gated
                # on this expert's B-phase retiring.
                w1t = w12pool.tile([128, EW], BF16, tag="w1t")
                w2t = w12pool.tile([128, EW], BF16, tag="w2t")
                w3t = w3pool.tile([128, EW], BF16, tag="w3t")
                o = e * EW
                if first:
                    # ramp: split w1 so the first chain's fo=0 slice lands
                    # quickly
                    q = EW // 4
                    nc.sync.dma_start(w1t[:, :q], w1[:, o:o + q])
                    nc.sync.dma_start(w1t[:, q:], w1[:, o + q:o + EW])
                else:
                    nc.sync.dma_start(w1t[:], w1[:, o:o + EW])
                nc.sync.dma_start(w2t[:], w2[:, o:o + EW])
                nc.sync.dma_start(w3t[:], w3[:, o:o + EW])
                return w1t, w2t, w3t

            def phase_a(si):
                e, L, off = segs2[si]  # L here is the padded length
                if e not in wtiles:
                    wtiles[e] = load_expert(e, first=(si == 0))
                w1t, w2t, _ = wtiles[e]
                xs = xpool.tile([128, HT * SEG_MAX], BF16, tag="xs")
                if si == 0:
                    hh = (HT // 2) * L
                    nc.sync.dma_start(xs[:, :hh], xg[:, off:off + hh])
                    nc.sync.dma_start(xs[:, hh:HT * L], xg[:, off + hh:off + HT * L])
                else:
                    nc.sync.dma_start(xs[:, :HT * L], xg[:, off:off + HT * L])
                h = hpool.tile([128, FLT * SEG_MAX], BF16, tag="h")
                for fo in range(FLT):
                    pa = psA.tile([128, SEG_MAX], F32, tag="pa")
                    for t in range(HT):
                        nc.tensor.matmul(
                            pa[:, :L],
                            w1t[:, (fo * HT + t) * 128:(fo * HT + t + 1) * 128],
                            xs[:, t * L:(t + 1) * L],
                            start=(t == 0), stop=(t == HT - 1),
                        )
                    pb = psA.tile([128, SEG_MAX], F32, tag="pb")
                    for t in range(HT):
                        nc.tensor.matmul(
                            pb[:, :L],
                            w2t[:, (fo * HT + t) * 128:(fo * HT + t + 1) * 128],
                            xs[:, t * L:(t + 1) * L],
                            start=(t == 0), stop=(t == HT - 1),
                        )
                    sa = spool.tile([128, SEG_MAX], F32, tag="sa")
                    nc.scalar.activation(
                        sa[:, :L], pa[:, :L], mybir.ActivationFunctionType.Silu
                    )
                    nc.vector.tensor_mul(
                        h[:, fo * L:(fo + 1) * L], sa[:, :L], pb[:, :L]
                    )
                hsb[si] = h

            def phase_b(si):
                e, L, off = segs2[si]
                w3t = wtiles[e][2]
                h = hsb.pop(si)
                for t in range(HT):
                    py = psY.tile([128, SEG_MAX], F32, tag="py")
                    for fo in range(FLT):
                        nc.tensor.matmul(
                            py[:, :L],
                            w3t[:, (fo * HT + t) * 128:(fo * HT + t + 1) * 128],
                            h[:, fo * L:(fo + 1) * L],
                            start=(fo == 0), stop=(fo == FLT - 1),
                        )
                    yt = ypool.tile([128, SEG_MAX], F32, tag="yt")
                    nc.vector.tensor_copy(yt[:, :L], py[:, :L])
                    nc.sync.dma_start(yT[:, off + t * L:off + (t + 1) * L], yt[:, :L])

            # segment offsets into the flat xg/yT streams (padded lengths)
            segs2 = []
            off = 0
            for e, _, Lp in segs:
                segs2.append((e, Lp, off))
                off += HT * Lp
            hsb = {}

            # software pipeline: A(0), A(1), B(0), A(2), B(1), ...
            n = len(segs2)
            phase_a(0)
            for si in range(1, n):
                phase_a(si)
                phase_b(si - 1)
            phase_b(n - 1)

    nc.compile()
    return nc


def _get_compiled(segs):
    if segs not in _COMPILED:
        _COMPILED[segs] = _build(segs)
    return _COMPILED[segs]


def kernel(hidden_states, selected_experts, routing_weights, w1, w2, w3):
    global LAST_EXEC_NS, LAST_RESULTS
    import ml_dtypes
    from concourse.bass_utils import run_bass_kernel_spmd

    BF = ml_dtypes.bfloat16
    hs = np.ascontiguousarray(np.asarray(hidden_states), dtype=np.float32)
    sel = np.asarray(selected_experts)
    rw = np.ascontiguousarray(np.asarray(routing_weights), dtype=np.float32)
    w1 = np.asarray(w1, dtype=np.float32)
    w2 = np.asarray(w2, dtype=np.float32)
    w3 = np.asarray(w3, dtype=np.float32)

    T = hs.shape[0]
    K = sel.shape[1]
    assert hs.shape[1] == H and w1.shape == (E, H, F) and w3.shape == (E, F, H)

    # host routing: gate[t, e] = sum_k rw[t, k] * (sel[t, k] == e)
    gate = np.zeros((T, E), np.float32)
    member = np.zeros((T, E), bool)
    tix = np.arange(T)
    for k in range(K):
        np.add.at(gate, (tix, sel[:, k]), rw[:, k])
        member[tix, sel[:, k]] = True
    idx = [np.nonzero(member[:, e])[0] for e in range(E)]
    counts = np.array([len(i) for i in idx])

    segs = _plan_segments(counts)
    tot = sum(Lp for _, _, Lp in segs)
    XW = HT * tot

    # pack the token stream: expert-pure segments, transposed to
    # [128 (h-within-tile), HT, Lp] and flattened per partition; pad
    # columns stay zero
    xr = hs.astype(BF)
    xgf = np.zeros((128, XW), BF)
    seg_meta = []  # (expert, token_indices, col_offset, padded_len)
    used = {e: 0 for e in range(E)}
    off = 0
    for e, L, Lp in segs:
        tt = idx[e][used[e]:used[e] + L]
        used[e] += L
        blk = xr[tt].reshape(L, HT, 128).transpose(2, 1, 0)  # [128, HT, L]
        xgf[:, off:off + HT * Lp].reshape(128, HT, Lp)[:, :, :L] = blk
        seg_meta.append((e, tt, off, Lp))
        off += HT * Lp

    # per-core F-sliced weights, bf16, per-partition-contiguous
    EW = FLT * HT * 128
    in_maps = []
    wz0 = np.zeros((128, 256), BF)
    for c in range(NC):
        fsl = slice(c * FL, (c + 1) * FL)
        w1p = np.empty((128, E * EW), BF)
        w2p = np.empty((128, E * EW), BF)
        w3p = np.empty((128, E * EW), BF)
        for e in range(E):
            # stat[p, (fo*HT+t)*128+j] = w1[e][t*128+p, c*FL+fo*128+j]
            w1p[:, e * EW:(e + 1) * EW] = (
                w1[e][:, fsl].astype(BF)
                .reshape(HT, 128, FLT, 128).transpose(1, 2, 0, 3)
                .reshape(128, EW)
            )
            w2p[:, e * EW:(e + 1) * EW] = (
                w2[e][:, fsl].astype(BF)
                .reshape(HT, 128, FLT, 128).transpose(1, 2, 0, 3)
                .reshape(128, EW)
            )
            # stat[p, (fo*HT+t)*128+j] = w3[e][c*FL+fo*128+p, t*128+j]
            w3p[:, e * EW:(e + 1) * EW] = (
                w3[e][fsl, :].astype(BF)
                .reshape(FLT, 128, HT, 128).transpose(1, 0, 2, 3)
                .reshape(128, EW)
            )
        in_maps.append({"xg": xgf, "w1": w1p, "w2": w2p, "w3": w3p, "wz": wz0})

    if TRACE:
        _ensure_ntff_hook()
    nc = _get_compiled(segs)
    res = run_bass_kernel_spmd(
        nc, in_maps, core_ids=list(range(NC)),
        trace=TRACE, trace_cores=(list(range(NC)) if TRACE else None),
    )
    if TRACE:
        LAST_EXEC_NS = res.exec_time_ns
        LAST_RESULTS = res

    # sum the 8 F-slice partials, then gate + scatter-add on the host
    ysum = res.results[0]["yT"].astype(np.float64)
    for c in range(1, NC):
        ysum += res.results[c]["yT"]
    ysum = ysum.astype(np.float32)

    out = np.zeros((T, H), np.float32)
    for e, tt, off, Lp in seg_meta:
        L = len(tt)
        y = (
            ysum[:, off:off + HT * Lp].reshape(128, HT, Lp)[:, :, :L]
            .transpose(2, 1, 0).reshape(L, H)
        )
        out[tt] += gate[tt, e:e + 1] * y
    return out
